# revision 1
# baseline (speedup 1.0000x reference)
"""CBOW negative-sampling loss on 8 Trainium2 NeuronCores.

Iteration-1 fallback (verified 2785321 ns on HW): f32 tables, NG=1024,
4 SWDGE queues, DVE multiply+reduce dots. Restore over kernel.py if the
bf16 variant regresses.
"""

import os
import sys

import numpy as np

if "/opt/trn_rl_repo" not in sys.path:
    sys.path.insert(0, "/opt/trn_rl_repo")

VOCAB = 200000
D = 128
B = 32768
C = 8
K = 5
NCORES = 8
P = 128

GB = B // NCORES            # groups per core (4096)
W = 32768                   # vocab window size (int16 range)
NW = (VOCAB + W - 1) // W   # 7 windows
NG = 1024                   # indices per dma_gather
SEG = NG // P               # 8 rows per partition per chunk

NEG_N = GB * C * K          # 163840
CTX_N = GB * C              # 32768
POS_N = GB                  # 4096


def _chunks_for(total):
    out = []
    for w in range(NW):
        frac = (min(VOCAB, (w + 1) * W) - w * W) / VOCAB
        mean = total * frac
        sd = (total * frac * (1 - frac)) ** 0.5
        out.append(max(1, int(np.ceil((mean + 8 * sd) / NG))))
    return out


NEG_CH = _chunks_for(NEG_N)     # per-window chunk counts
CTX_CH = _chunks_for(CTX_N)
POS_CH = _chunks_for(POS_N)
NEG_TOT = sum(NEG_CH)
CTX_TOT = sum(CTX_CH)
POS_TOT = sum(POS_CH)
TOT_CH = NEG_TOT + CTX_TOT + POS_TOT

VROWS = 4224                # V table rows: 0..4095 real, 4096 zero, 4223 dump
VZERO = 4096
VDUMP = 4223

_CACHE = {}


def _build_program():
    import concourse.bass as bass
    import concourse.mybir as mybir
    from concourse import bacc
    from concourse.library_config import mlp
    from concourse.tile import TileContext

    f32 = mybir.dt.float32
    i16 = mybir.dt.int16

    nc = bacc.Bacc("TRN2", num_swdge_queues=4)
    uw = nc.declare_dram_parameter("u_weights", [VOCAB, D], f32, isOutput=False)
    vw = nc.declare_dram_parameter("v_weights", [VOCAB, D], f32, isOutput=False)
    idxd = nc.declare_dram_parameter(
        "idx_all", [P, TOT_CH * 2 * (NG // 16)], i16, isOutput=False
    )
    lp = nc.declare_dram_parameter("loss_parts", [P, 2], f32, isOutput=True)

    IW = NG // 16
    NCOMP = NEG_TOT + POS_TOT
    NZ = VROWS // P

    with TileContext(nc) as tc:
        with (
            tc.tile_pool(name="fix", bufs=1) as fixp,
            tc.tile_pool(name="vtd", bufs=1, space="DRAM") as vtp,
            tc.tile_pool(name="ct", bufs=6) as ctp,
            tc.tile_pool(name="eb", bufs=6) as ebp,
            tc.tile_pool(name="sm", bufs=6) as smp,
        ):
            nc.gpsimd.load_library(mlp)
            vtab_t = vtp.tile([VROWS, D], f32)
            vtab = vtab_t[:]
            idxt = fixp.tile([P, TOT_CH * 2 * IW], i16)
            nc.sync.dma_start(out=idxt[:], in_=idxd[:])
            zt = fixp.tile([P, NZ * D], f32)
            nc.vector.memset(zt[:], 0.0)
            acc = fixp.tile([P, 2], f32)
            nc.vector.memset(acc[:], 0.0)

            def idx_ap(chunk_i, which):
                off = (chunk_i * 2 + which) * IW
                return idxt[:, off:off + IW]

            # zero the V table (Tile tracks the DRAM pool tile deps)
            for i in range(NZ):
                nc.gpsimd.dma_start(
                    out=vtab[i * P:(i + 1) * P, :],
                    in_=zt[:, i * D:(i + 1) * D],
                )

            ch = 0
            # context phase: gather rows, scatter-add into vtab by group id
            for kk in range(CTX_TOT):
                et = ctp.tile([P, SEG, D], f32, tag="ce")
                nc.gpsimd.dma_gather(
                    et[:], vw[:], idx_ap(ch, 0), NG, NG, D,
                    queue_num=1 + kk % 3,
                )
                nc.gpsimd.dma_scatter_add(
                    vtab, et[:], idx_ap(ch, 1), NG, NG, D,
                )
                ch += 1

            # negatives then positives; vex gathers read vtab -> gate on sc
            for k in range(NCOMP):
                is_pos = k >= NEG_TOT
                src = uw if is_pos else vw
                et = ebp.tile([P, SEG, D], f32, tag="emb")
                vt = ebp.tile([P, SEG, D], f32, tag="vex")
                nc.gpsimd.dma_gather(
                    et[:], src[:], idx_ap(ch, 0), NG, NG, D,
                    queue_num=(2 * k) % 4,
                )
                nc.gpsimd.dma_gather(
                    vt[:], vtab, idx_ap(ch, 1), NG, NG, D,
                    queue_num=(2 * k + 1) % 4,
                )
                ch += 1

                st = smp.tile([P, SEG], f32, tag="sco")
                nc.vector.tensor_tensor(
                    out=et[:], in0=et[:], in1=vt[:], op=mybir.AluOpType.mult,
                )
                nc.vector.tensor_reduce(
                    out=st[:], in_=et[:], axis=mybir.AxisListType.X,
                    op=mybir.AluOpType.add,
                )
                nc.vector.tensor_scalar(
                    out=st[:], in0=st[:], scalar1=10.0, scalar2=-10.0,
                    op0=mybir.AluOpType.min, op1=mybir.AluOpType.max,
                )
                spt = smp.tile([P, SEG], f32, tag="sp")
                bt = smp.tile([P, 1], f32, tag="blk")
                nc.scalar.activation(
                    out=spt[:], in_=st[:],
                    func=mybir.ActivationFunctionType.Exp,
                    scale=-1.0 if is_pos else 1.0,
                )
                nc.scalar.activation(
                    out=spt[:], in_=spt[:],
                    func=mybir.ActivationFunctionType.Ln, bias=1.0,
                    accum_out=bt[:],
                )
                col = 0 if is_pos else 1
                nc.vector.tensor_tensor(
                    out=acc[:, col:col + 1], in0=acc[:, col:col + 1],
                    in1=bt[:], op=mybir.AluOpType.add,
                )

            nc.sync.dma_start(out=lp[:], in_=acc[:])
    nc.finalize()
    return nc


def _window_sort(idx, gid, chunks):
    """Sort (idx, gid) by vocab window; pad each window to chunks[w]*NG.
    Returns wrapped int16 arrays [nch, P, NG//16] x2 and the pad count."""
    order = np.argsort(idx // W, kind="stable")
    si, sg = idx[order], gid[order]
    wi = si // W
    out_i, out_g = [], []
    npad = 0
    for w in range(NW):
        m = wi == w
        li = (si[m] - w * W).astype(np.int16)
        lg = sg[m].astype(np.int16)
        cap = chunks[w] * NG
        if len(li) > cap:
            raise RuntimeError(f"window {w} overflow: {len(li)} > {cap}")
        pad = cap - len(li)
        npad += pad
        li = np.concatenate([li, np.zeros(pad, np.int16)])
        lg = np.concatenate([lg, np.full(pad, VZERO, np.int16)])
        out_i.append(li)
        out_g.append(lg)
    fi = np.concatenate(out_i).reshape(-1, NG)
    fg = np.concatenate(out_g).reshape(-1, NG)

    def wrap(a):  # [nch, NG] -> [nch, P, NG//16]
        w16 = a.reshape(a.shape[0], NG // 16, 16)
        return np.tile(w16.transpose(0, 2, 1), (1, 8, 1)).astype(np.int16)

    return wrap(fi), wrap(fg), npad


def _prep_core(pos_u, pos_v, neg_v, core):
    sl = slice(core * GB, (core + 1) * GB)
    gids = np.arange(GB, dtype=np.int64)
    negf = neg_v.reshape(B, C * K)[sl].astype(np.int64)
    ctxf = pos_v.reshape(B, C)[sl].astype(np.int64)
    posf = pos_u.reshape(B, C)[sl][:, 0].astype(np.int64)

    neg_i, neg_g, npad_n = _window_sort(
        negf.ravel(), np.repeat(gids, C * K), NEG_CH)
    ctx_i, ctx_g, _ = _window_sort(
        ctxf.ravel(), np.repeat(gids, C), CTX_CH)
    # ctx companion is the scatter target: pads go to the dump row
    ctx_g[ctx_g == VZERO] = VDUMP
    pos_i, pos_g, npad_p = _window_sort(posf, gids, POS_CH)

    # interleave [idx, gid] per chunk in program order: ctx, neg, pos
    parts = []
    for i_arr, g_arr in ((ctx_i, ctx_g), (neg_i, neg_g), (pos_i, pos_g)):
        inter = np.empty((i_arr.shape[0] * 2, P, NG // 16), np.int16)
        inter[0::2] = i_arr
        inter[1::2] = g_arr
        parts.append(inter)
    allc = np.concatenate(parts, axis=0)            # [TOT_CH*2, P, 64]
    packed = np.ascontiguousarray(
        allc.transpose(1, 0, 2).reshape(P, TOT_CH * 2 * (NG // 16))
    )
    return packed, npad_n, npad_p


def _prep_indices(pos_u, pos_v, neg_v):
    out = []
    for c in range(NCORES):
        out.append(_prep_core(pos_u, pos_v, neg_v, c))
    return out


def _prep_tables(u_weights, v_weights):
    u_w = np.ascontiguousarray(np.asarray(u_weights, dtype=np.float32))
    v_w = np.ascontiguousarray(np.asarray(v_weights, dtype=np.float32))
    return u_w, v_w


def kernel(u_weights, v_weights, pos_u, pos_v, neg_v, context_size):
    from concourse.bass_utils import run_bass_kernel_spmd

    assert int(context_size) == C
    u_w, v_w = _prep_tables(u_weights, v_weights)
    pos_u = np.asarray(pos_u)
    pos_v = np.asarray(pos_v)
    neg_v = np.asarray(neg_v)

    if "nc" not in _CACHE:
        _CACHE["nc"] = _build_program()
    nc = _CACHE["nc"]

    prep = _prep_indices(pos_u, pos_v, neg_v)
    in_maps = [
        {"u_weights": u_w, "v_weights": v_w, "idx_all": prep[c][0]}
        for c in range(NCORES)
    ]
    res = run_bass_kernel_spmd(nc, in_maps, list(range(NCORES)))
    LN2 = float(np.log(2.0))
    total = np.float64(0.0)
    for c in range(NCORES):
        parts = res.results[c]["loss_parts"].astype(np.float64)
        pos_sum = parts[:, 0].sum() - prep[c][2] * LN2
        neg_sum = parts[:, 1].sum() - prep[c][1] * LN2
        total += pos_sum + neg_sum / (C * K)
    return np.float32(total / B)



# revision 2
# speedup vs baseline: 1.1087x; 1.1087x over previous
"""CBOW negative-sampling loss on 8 Trainium2 NeuronCores — v2.

Structure (per core, data-parallel over the B=32768 groups):
  - bf16 copies of both embedding tables in DRAM, viewed [25000, 8, 128]
    so gathers address super-rows (int16-safe) with elem_step=1024 and a
    per-class base offset: row r = 8*(r//8) + r%8.
  - ctx phase: 8 rounds x 8 classes of gather + dma_scatter_add, each
    round into its OWN vtab copy (every gid appears exactly once per
    round -> no same-address RMW races; rounds are independent).  vtab
    rows are padded to 512B so no two gids share an RMW granule.
  - V = sum of the 8 partial vtabs (DVE adds in SBUF), stored back to a
    final vtab in DRAM.
  - pos/neg phase: row gather (u/v table) + vex gather (V by gid),
    bf16 multiply + reduce -> f32 scores, clip, Softplus on the scalar
    engine with accumulation; pads gather VZERO so they contribute
    exactly softplus(0)=ln2, subtracted on the host.
"""

import sys

import numpy as np

if "/opt/trn_rl_repo" not in sys.path:
    sys.path.insert(0, "/opt/trn_rl_repo")

VOCAB = 200000
D = 128
B = 32768
C = 8
K = 5
NCORES = 8
P = 128

GB = B // NCORES            # 4096 groups per core
S8 = VOCAB // 8             # 25000 super-rows per mod-8 class
NCLS = 8

VROWS = 4224                # vtab rows: 0..4095 groups, 4096 zero, 4223 dump
VZERO = 4096
VDUMP = 4223
VSTEP = 256                 # vtab row stride in bf16 elems (512B padded)

CTX_NG = 640                # per (round, class) ctx chunk
POS_NG = 640                # per class pos chunk
NEG_NGS = [1024] * 21   # per class neg chunks (cap 21504); NG>1024 gathers crash HW
NEG_CAP = sum(NEG_NGS)
PREFETCH = 24
NQUEUES = 4


def _chunk_table():
    chunks = []
    for t in range(C):
        for u in range(NCLS):
            chunks.append(("ctx", u, CTX_NG, t))
    for u in range(NCLS):
        chunks.append(("pos", u, POS_NG, 0))
    for u in range(NCLS):
        for ng in NEG_NGS:
            chunks.append(("neg", u, ng, 0))
    return chunks


CHUNKS = _chunk_table()
IDX_W = sum(2 * ng // 16 for (_, _, ng, _) in CHUNKS)

_CACHE = {}


def _build_program():
    import concourse.mybir as mybir
    from concourse import bacc
    from concourse.library_config import mlp
    from concourse.tile import TileContext

    f32 = mybir.dt.float32
    bf16 = mybir.dt.bfloat16
    i16 = mybir.dt.int16

    nc = bacc.Bacc("TRN2", num_swdge_queues=max(NQUEUES, 1))
    ub = nc.declare_dram_parameter("u_bf", [S8, NCLS, D], bf16, isOutput=False)
    vb = nc.declare_dram_parameter("v_bf", [S8, NCLS, D], bf16, isOutput=False)
    idxd = nc.declare_dram_parameter("idx_all", [P, IDX_W], i16, isOutput=False)
    lp = nc.declare_dram_parameter("loss_parts", [P, 2], f32, isOutput=True)

    q_state = [0]

    def nq():
        q = q_state[0]
        q_state[0] = (q + 1) % NQUEUES
        return q

    with TileContext(nc) as tc:
        with (
            tc.tile_pool(name="fix", bufs=1) as fixp,
            tc.tile_pool(name="vtd", bufs=1, space="DRAM") as vtp,
            tc.tile_pool(name="ct", bufs=14) as ctp,
            tc.tile_pool(name="row", bufs=PREFETCH + 3) as rowp,
            tc.tile_pool(name="vex", bufs=12) as vexp,
            tc.tile_pool(name="sum", bufs=2) as sump,
            tc.tile_pool(name="sm", bufs=8) as smp,
        ):
            nc.gpsimd.load_library(mlp)
            vt8_t = vtp.tile([C, VROWS, VSTEP], bf16)
            vtf_t = vtp.tile([VROWS, VSTEP], bf16)
            idxt = fixp.tile([P, IDX_W], i16)
            nc.sync.dma_start(out=idxt[:], in_=idxd[:])
            zt = fixp.tile([P, VROWS], bf16)
            nc.vector.memset(zt[:], 0.0)
            # zero the read columns (0:128) of all 8 partial vtabs + final
            for t in range(C):
                nc.sync.dma_start(out=vt8_t[t, :, 0:D], in_=zt[:])
            nc.sync.dma_start(out=vtf_t[:, 0:D], in_=zt[:])
            acc = fixp.tile([P, 2], f32)
            nc.vector.memset(acc[:], 0.0)
            # persistent exp-score buffers: all Exp ops batch together so the
            # activation-table load is hoisted (no per-chunk reloads), and a
            # single Ln(1+x) pass per column finishes -log sigmoid.
            n_pos_seg = sum(ng // P for (ph, _, ng, _) in CHUNKS if ph == "pos")
            n_neg_seg = sum(ng // P for (ph, _, ng, _) in CHUNKS if ph == "neg")
            pbuf = fixp.tile([P, n_pos_seg], f32)
            nbuf = fixp.tile([P, n_neg_seg], f32)

            # per-chunk idx access: [idxA, idxB] interleaved in chunk order
            offs = []
            off = 0
            for (_, _, ng, _) in CHUNKS:
                offs.append(off)
                off += 2 * (ng // 16)

            def idx_ap(ci, which):
                ph, u, ng, t = CHUNKS[ci]
                iw = ng // 16
                o = offs[ci] + which * iw
                return idxt[:, o:o + iw]

            ctx_chunks = [i for i, c in enumerate(CHUNKS) if c[0] == "ctx"]
            pos_chunks = [i for i, c in enumerate(CHUNKS) if c[0] == "pos"]
            neg_chunks = [i for i, c in enumerate(CHUNKS) if c[0] == "neg"]

            # ---- ctx phase: fully concurrent, race-free by construction
            for ci in ctx_chunks:
                ph, u, ng, t = CHUNKS[ci]
                seg = ng // P
                et = ctp.tile([P, seg, D], bf16, tag="ce")
                nc.gpsimd.dma_gather(
                    et[:], vb[:, u, :], idx_ap(ci, 0), ng, ng, D,
                    elem_step=NCLS * D, queue_num=nq(),
                )
                nc.gpsimd.dma_scatter_add(
                    vt8_t[t, :, 0:D], et[:], idx_ap(ci, 1), ng, ng, D,
                    elem_step=VSTEP, queue_num=nq(),
                )

            # ---- prefetch first neg row gathers (independent of vtab)
            comp = pos_chunks + neg_chunks
            row_tiles = {}

            def emit_row_gather(ci):
                ph, u, ng, t = CHUNKS[ci]
                seg = ng // P
                src = ub if ph == "pos" else vb
                et = rowp.tile([P, seg, D], bf16, tag="emb")
                nc.gpsimd.dma_gather(
                    et[:], src[:, u, :], idx_ap(ci, 0), ng, ng, D,
                    elem_step=NCLS * D, queue_num=nq(),
                )
                row_tiles[ci] = et

            for ci in comp[:PREFETCH]:
                emit_row_gather(ci)

            # ---- V = sum of partial vtabs
            vsum = fixp.tile([P, 33, D], bf16)
            lt0 = sump.tile([P, 33, D], bf16, tag="ld")
            nc.sync.dma_start(out=lt0[:], in_=vt8_t[0, :, 0:D])
            lt1 = sump.tile([P, 33, D], bf16, tag="ld")
            nc.sync.dma_start(out=lt1[:], in_=vt8_t[1, :, 0:D])
            nc.vector.tensor_tensor(
                out=vsum[:], in0=lt0[:], in1=lt1[:], op=mybir.AluOpType.add)
            for t in range(2, C):
                lt = sump.tile([P, 33, D], bf16, tag="ld")
                nc.sync.dma_start(out=lt[:], in_=vt8_t[t, :, 0:D])
                nc.vector.tensor_tensor(
                    out=vsum[:], in0=vsum[:], in1=lt[:],
                    op=mybir.AluOpType.add)
            nc.sync.dma_start(out=vtf_t[:, 0:D], in_=vsum[:])

            # ---- pos + neg compute chunks
            pos_off = 0
            neg_off = 0
            for i, ci in enumerate(comp):
                ph, u, ng, t = CHUNKS[ci]
                seg = ng // P
                if i + PREFETCH < len(comp):
                    emit_row_gather(comp[i + PREFETCH])
                vt = vexp.tile([P, seg, D], bf16, tag="vex")
                nc.gpsimd.dma_gather(
                    vt[:], vtf_t[:, 0:D], idx_ap(ci, 1), ng, ng, D,
                    elem_step=VSTEP, queue_num=nq(),
                )
                et = row_tiles.pop(ci)
                nc.vector.tensor_tensor(
                    out=et[:], in0=et[:], in1=vt[:], op=mybir.AluOpType.mult)
                st = smp.tile([P, seg], f32, tag="sco")
                nc.vector.tensor_reduce(
                    out=st[:], in_=et[:], axis=mybir.AxisListType.X,
                    op=mybir.AluOpType.add)
                nc.vector.tensor_scalar(
                    out=st[:], in0=st[:], scalar1=10.0, scalar2=-10.0,
                    op0=mybir.AluOpType.min, op1=mybir.AluOpType.max)
                if ph == "pos":
                    dst = pbuf[:, pos_off:pos_off + seg]
                    pos_off += seg
                    scale = -1.0
                else:
                    dst = nbuf[:, neg_off:neg_off + seg]
                    neg_off += seg
                    scale = 1.0
                nc.scalar.activation(
                    out=dst, in_=st[:],
                    func=mybir.ActivationFunctionType.Exp, scale=scale)

            # ---- final: -log sigmoid = ln(1 + exp(+-s)), summed per column
            bt0 = smp.tile([P, 1], f32, tag="blk")
            nc.scalar.activation(
                out=pbuf[:], in_=pbuf[:],
                func=mybir.ActivationFunctionType.Ln, bias=1.0,
                accum_out=bt0[:])
            nc.vector.tensor_tensor(
                out=acc[:, 0:1], in0=acc[:, 0:1], in1=bt0[:],
                op=mybir.AluOpType.add)
            bt1 = smp.tile([P, 1], f32, tag="blk")
            nc.scalar.activation(
                out=nbuf[:], in_=nbuf[:],
                func=mybir.ActivationFunctionType.Ln, bias=1.0,
                accum_out=bt1[:])
            nc.vector.tensor_tensor(
                out=acc[:, 1:2], in0=acc[:, 1:2], in1=bt1[:],
                op=mybir.AluOpType.add)

            nc.sync.dma_start(out=lp[:], in_=acc[:])
    nc.finalize()
    return nc


def _wrap(a):
    """[ng] int array -> [P, ng//16] int16 (16-partition wrap, replicated 8x)."""
    ng = a.shape[0]
    w16 = a.reshape(ng // 16, 16)
    return np.ascontiguousarray(
        np.tile(w16.T, (8, 1))).astype(np.int16)


def _class_split(rows, comp, cap, pad_comp):
    """Split (row, companion) pairs by row%8; sort each class by super-row;
    pad each class to cap with (super 0, pad_comp). Returns per-class
    (supers[cap], comps[cap]) plus total pad count."""
    u = rows % NCLS
    s = rows // NCLS
    out = []
    npad = 0
    for cls in range(NCLS):
        m = u == cls
        si = s[m]
        gi = comp[m]
        if len(si) > cap:
            raise RuntimeError(f"class {cls} overflow: {len(si)} > {cap}")
        order = np.argsort(si, kind="stable")
        si = si[order]
        gi = gi[order]
        pad = cap - len(si)
        npad += pad
        si = np.concatenate([si, np.zeros(pad, np.int64)])
        gi = np.concatenate([gi, np.full(pad, pad_comp, np.int64)])
        out.append((si, gi))
    return out, npad


def _prep_core(pos_u, pos_v, neg_v, core):
    sl = slice(core * GB, (core + 1) * GB)
    gids = np.arange(GB, dtype=np.int64)
    ctxf = pos_v.reshape(B, C)[sl].astype(np.int64)
    posf = pos_u.reshape(B, C)[sl][:, 0].astype(np.int64)
    negf = neg_v.reshape(B, C * K)[sl].astype(np.int64)

    blocks = {}
    # ctx: round t = t-th context entry of each group
    for t in range(C):
        cls_list, _ = _class_split(ctxf[:, t], gids, CTX_NG, VDUMP)
        for u in range(NCLS):
            blocks[("ctx", u, t)] = cls_list[u]
    cls_list, npad_pos = _class_split(posf, gids, POS_NG, VZERO)
    for u in range(NCLS):
        blocks[("pos", u, 0)] = cls_list[u]
    cls_list, npad_neg = _class_split(
        negf.ravel(), np.repeat(gids, C * K), NEG_CAP, VZERO)
    neg_cuts = np.cumsum([0] + NEG_NGS)
    for u in range(NCLS):
        si, gi = cls_list[u]
        for j, ng in enumerate(NEG_NGS):
            blocks[("neg", u, j)] = (si[neg_cuts[j]:neg_cuts[j + 1]],
                                     gi[neg_cuts[j]:neg_cuts[j + 1]])

    parts = []
    neg_j = {}
    for (ph, u, ng, t) in CHUNKS:
        if ph == "neg":
            j = neg_j.get(u, 0)
            neg_j[u] = j + 1
            si, gi = blocks[("neg", u, j)]
        else:
            si, gi = blocks[(ph, u, t)]
        parts.append(_wrap(si))
        parts.append(_wrap(gi))
    packed = np.ascontiguousarray(np.concatenate(parts, axis=1))
    assert packed.shape == (P, IDX_W), packed.shape
    return packed, npad_pos, npad_neg


def _prep_tables(u_weights, v_weights):
    import ml_dtypes
    u_bf = np.ascontiguousarray(
        np.asarray(u_weights, dtype=np.float32).astype(ml_dtypes.bfloat16)
        .reshape(S8, NCLS, D))
    v_bf = np.ascontiguousarray(
        np.asarray(v_weights, dtype=np.float32).astype(ml_dtypes.bfloat16)
        .reshape(S8, NCLS, D))
    return u_bf, v_bf


def kernel(u_weights, v_weights, pos_u, pos_v, neg_v, context_size):
    from concourse.bass_utils import run_bass_kernel_spmd

    assert int(context_size) == C
    u_bf, v_bf = _prep_tables(u_weights, v_weights)
    pos_u = np.asarray(pos_u)
    pos_v = np.asarray(pos_v)
    neg_v = np.asarray(neg_v)

    if "nc" not in _CACHE:
        _CACHE["nc"] = _build_program()
    nc = _CACHE["nc"]

    prep = [_prep_core(pos_u, pos_v, neg_v, c) for c in range(NCORES)]
    in_maps = [
        {"u_bf": u_bf, "v_bf": v_bf, "idx_all": prep[c][0]}
        for c in range(NCORES)
    ]
    res = run_bass_kernel_spmd(nc, in_maps, list(range(NCORES)))
    LN2 = float(np.log(2.0))
    total = np.float64(0.0)
    for c in range(NCORES):
        parts = res.results[c]["loss_parts"].astype(np.float64)
        pos_sum = parts[:, 0].sum() - prep[c][1] * LN2
        neg_sum = parts[:, 1].sum() - prep[c][2] * LN2
        total += pos_sum + neg_sum / (C * K)
    return np.float32(total / B)
